# revision 28
# baseline (speedup 1.0000x reference)
"""Trainium2 Bass kernel for nn_Attention_8143257993917.

Multi-head attention (packed QKV + RoPE + additive bias + softmax + head_mask
+ o_proj), B=4, S=2048, D=1024, H=16 heads, fp32 I/O.

Sharding: 8 cores = 4 batches x 2 head-groups (tensor-parallel over heads).
Core c handles batch b = c // 2 and heads g*8..g*8+8 with g = c % 2.
Each core computes a partial output (its heads' contribution through o_proj);
the host sums the two partials per batch and adds o_b.

Device-side design (per core), v2:
- Transposed feature-major layouts throughout (no on-chip transposes):
    Q_T, K_T: [f, t]; RoPE via second projection with host-prerotated weights.
    V: [t, f] natural, ones-column appended -> PV matmul also emits softmax
    denominators (row 64 of ctx psum).
- Phase B (attention) per (qh-half, head-pair):
    scores for both heads of a pair land in ONE [128, 2x512] PSUM tile
    (kc, qc)-granular, double-buffered (4 banks) so scores of step s+1 fill
    while exp of step s drains.  attention bias is added IN PSUM via an
    identity matmul (fp16 identity stationary, raw fp16 bias moving) --
    keeps the PE ~95% busy (HAM clock-gate stays warm) and frees VectorE.
    One ScalarE exp per step covers both heads ([128, 1024], PSUM->SBUF,
    fp16 out, softmax shift -12).  PV lags one step; ctx accumulates in the
    other 4 PSUM banks.  Denominator reciprocal via [32,32] DRAM-roundtrip
    reshape (fp16), normalize on VectorE in fp16.
- bias tile for the current q-half stays SBUF-resident (one 4MB load per
  half instead of per head-pair).
- Matmul dtype fp16 (same PE rate as bf16, ~8x lower quantization error).
"""

import sys

sys.path.insert(0, "/opt/trn_rl_repo")

import numpy as np

_CACHE = {}

H = 16
HPC = 8  # heads per core
G = 2  # head groups


def build_nc(S=2048, D=1024):
    """Build + compile the per-core Bass program (same program on all cores)."""
    import concourse.bass as bass
    from concourse import bacc
    import concourse.mybir as mybir
    import concourse.tile as tile
    from concourse.masks import make_identity
    from concourse.tile_rust import add_dep_helper

    F32 = mybir.dt.float32
    F16 = mybir.dt.float16
    MT = F16
    AF = mybir.ActivationFunctionType

    P = 128
    DC = D // P          # d chunks (contraction for projections)
    KC = S // P          # k chunks (scores contraction)
    NQH = S // 2         # q-half size
    NQ = 512             # scores/PV q-chunk (one psum bank)
    FPC = HPC * 64       # features per core (= 512)
    FT = FPC // P        # f-tiles per tensor (= 4)
    NT = 512             # phase C t-chunk
    NTA = 512            # phase A t-chunk

    nc = bacc.Bacc("TRN2", target_bir_lowering=False, debug=False, num_devices=8)

    hT = nc.dram_tensor("hT", [D, S], MT, kind="ExternalInput")
    w4 = nc.dram_tensor("w4", [D, 2 * FPC], MT, kind="ExternalInput")
    b4 = nc.dram_tensor("b4", [2 * FPC], F32, kind="ExternalInput")
    permM = nc.dram_tensor("permM", [P, P], MT, kind="ExternalInput")
    wvT = nc.dram_tensor("wvT", [D, FPC], MT, kind="ExternalInput")
    bv = nc.dram_tensor("bv", [FPC], MT, kind="ExternalInput")
    cosr = nc.dram_tensor("cosr", [P, S], F16, kind="ExternalInput")
    sinr = nc.dram_tensor("sinr", [P, S], F16, kind="ExternalInput")
    biasT = nc.dram_tensor("biasT", [S, S], F16, kind="ExternalInput")
    owT = nc.dram_tensor("owT", [FPC, D], MT, kind="ExternalInput")
    outT = nc.dram_tensor("outT", [D, S], F32, kind="ExternalOutput")

    hT_r = hT.ap().rearrange("(o p) t -> p o t", p=P)
    w4_r = w4.ap().rearrange("(o p) f -> p o f", p=P)
    wv_r = wvT.ap().rearrange("(o p) f -> p o f", p=P)
    ow_r = owT.ap().rearrange("(o p) f -> p o f", p=P)
    b4_r = b4.ap().rearrange("(o p) -> p o", p=P)
    bias_r = biasT.ap().rearrange("(kc p) q -> p kc q", p=P)

    with tile.TileContext(nc) as tc:
        with (
            tc.tile_pool(name="cst", bufs=1) as cst,
            tc.tile_pool(name="pAB", bufs=1) as pAB,
            tc.tile_pool(name="dram", bufs=4, space="DRAM") as dpool,
        ):
            identF = cst.tile([P, P], MT)
            make_identity(nc, identF)
            ones1 = cst.tile([1, P], MT)
            nc.vector.memset(ones1[:], 1.0)
            b4_sb = cst.tile([P, 2 * FPC // P], F32)
            nc.gpsimd.dma_start(b4_sb[:], b4_r)
            bv_sb = cst.tile([1, FPC], MT)
            eshift = cst.tile([P, 1], F32)
            nc.vector.memset(eshift[:], -12.0)
            nc.gpsimd.dma_start(bv_sb[:], bv.ap()[None, :])
            perm_sb = cst.tile([P, P], MT)
            nc.gpsimd.dma_start(perm_sb[:], permM.ap())

            # persistent phase A->B products
            qk_sb = pAB.tile([P, 2 * FT, S], MT)          # slots: Q ft 0..FT-1, K ft FT..2FT-1
            v_sb = pAB.tile([P, KC, HPC, 66], MT)          # col 64 = ones
            bias_sb = pAB.tile([P, 2, KC, NQH], F16)       # both q-halves

            nc.vector.memset(v_sb[:, :, :, 64:65], 1.0)

            # ---------------- Phase A: projections + rope ----------------
            with (
                tc.tile_pool(name="pA", bufs=1) as pA,
                tc.tile_pool(name="pAw", bufs=2) as pAw,
                tc.tile_pool(name="psA", bufs=2, space="PSUM") as ppsA,
            ):
                for half in range(2):
                    tsl = slice(half * NQH, (half + 1) * NQH)
                    cos_sb = pA.tile([P, NQH], F16, tag="cos", bufs=2)
                    nc.gpsimd.dma_start(cos_sb[:], cosr.ap()[:, tsl])
                    sin_sb = pA.tile([P, NQH], F16, tag="sin", bufs=2)
                    nc.gpsimd.dma_start(sin_sb[:], sinr.ap()[:, tsl])
                    h_sb = pA.tile([P, DC, NQH], MT, tag="hT", bufs=2)
                    for dc in range(DC):
                        nc.sync.dma_start(h_sb[:, dc], hT_r[:, dc, tsl])

                    # Q/K projections; RoPE rotate_half via a permutation
                    # matmul on the biased projection (rot(q + bq) includes
                    # the rotated bias automatically)
                    for qk in range(2):            # 0 = Q, 1 = K
                        for ft in range(FT):
                            fcol = qk * FPC + ft * P
                            wa = pAw.tile([P, DC, P], MT, tag="wA")
                            nc.sync.dma_start(wa[:], w4_r[:, :, fcol:fcol + P])
                            bcol = qk * FT + ft
                            for tq in range(NQH // NTA):
                                qsl = slice(tq * NTA, (tq + 1) * NTA)
                                pa = ppsA.tile([P, NTA], F32, tag="pa", name="pa")
                                for dc in range(DC):
                                    nc.tensor.matmul(pa[:], wa[:, dc], h_sb[:, dc, qsl],
                                                     start=(dc == 0), stop=(dc == DC - 1))
                                ub = pAw.tile([P, NTA], F16, tag="ub")
                                nc.scalar.activation(ub[:], pa[:], AF.Identity,
                                                     bias=b4_sb[:, bcol:bcol + 1])
                                pr = ppsA.tile([P, NTA], F32, tag="pr", name="pr")
                                nc.tensor.matmul(pr[:], perm_sb[:], ub[:],
                                                 start=True, stop=True)
                                tca = pAw.tile([P, NTA], F16, tag="tca")
                                nc.vector.tensor_mul(tca[:], ub[:], cos_sb[:, qsl])
                                tcb = pAw.tile([P, NTA], F16, tag="tcb")
                                nc.vector.tensor_mul(tcb[:], pr[:], sin_sb[:, qsl])
                                dst = qk_sb[:, qk * FT + ft, half * NQH + tq * NTA:
                                            half * NQH + (tq + 1) * NTA]
                                nc.vector.tensor_add(dst, tca[:], tcb[:])

                    # big attention-bias load for this half rides the
                    # Activation HWDGE queue, emitted only now so its HBM
                    # traffic does not starve the startup-critical loads
                    nc.scalar.dma_start(bias_sb[:, half], bias_r[:, :, tsl])

                    # V for this half: t-tiles within half
                    wvs = pA.tile([P, DC, FPC], MT, tag="wV")
                    nc.gpsimd.dma_start(wvs[:], wv_r)
                    for tt in range(NQH // P):
                        gt = half * (NQH // P) + tt            # global t-tile = k-chunk
                        pv = ppsA.tile([P, FPC], F32, tag="pv", name="pv")
                        for dc in range(DC):
                            nc.tensor.matmul(pv[:], h_sb[:, dc, tt * P:(tt + 1) * P],
                                             wvs[:, dc], start=(dc == 0), stop=False)
                        nc.tensor.matmul(pv[:], ones1[:], bv_sb[:], start=False, stop=True)
                        nc.vector.tensor_copy(v_sb[:, gt, :, 0:64], pv[:])

            with tc.tile_pool(name="pBC", bufs=1) as pBC:
                ctxT = pBC.tile([P, FT, S], MT)            # normalized ctx, f-major
                ow_sb = pBC.tile([P, FT, D], MT)
                nc.sync.dma_start(ow_sb[:], ow_r)

                # ---------------- Phase B: attention ----------------
                with (
                    tc.tile_pool(name="pB", bufs=2) as pB,
                    tc.tile_pool(name="psB", bufs=1, space="PSUM") as ppsB,
                ):
                    for qh in range(2):
                        qoff = qh * NQH
                        for hp in range(HPC // 2):
                            cps = []
                            for i in range(2):
                                ct = ppsB.tile([P, NQH], F32, tag=f"ctx{i}",
                                               name=f"ctx{i}")
                                cps.append(ct[:65, :])
                            prev = None
                            for kc in range(KC):
                                for qc in range(2):
                                    csl = slice(qc * NQ, (qc + 1) * NQ)
                                    psS = ppsB.tile([P, 2, NQ], F32, tag="ps",
                                                    name="psS", bufs=2)
                                    prev_mm = None
                                    for hi in range(2):
                                        h = 2 * hp + hi
                                        base = 64 * (h % 2)
                                        ft = h // 2
                                        ksl = qk_sb[base:base + 64, FT + ft,
                                                    kc * P:(kc + 1) * P]
                                        qsl = qk_sb[base:base + 64, ft,
                                                    qoff + qc * NQ:
                                                    qoff + (qc + 1) * NQ]
                                        mm = nc.tensor.matmul(psS[:, hi], ksl,
                                                              qsl, start=True,
                                                              stop=False)
                                        if prev_mm is not None:
                                            add_dep_helper(
                                                mm.ins, prev_mm.ins, sync=False,
                                                reason="scores row-group pairing")
                                        prev_mm = mm
                                    for hi in range(2):
                                        nc.tensor.matmul(psS[:, hi], identF[:],
                                                         bias_sb[:, qh, kc, csl],
                                                         start=False, stop=True)
                                    u2 = pB.tile([P, 2, NQ], F16, tag="u2", bufs=3)
                                    nc.scalar.activation(u2[:], psS[:], AF.Exp,
                                                         bias=eshift[:])
                                    # software-pipeline: PV lags one step so the
                                    # PE never waits on this step's exp
                                    if prev is not None:
                                        pkc, pqc, pu = prev
                                        psl = slice(pqc * NQ, (pqc + 1) * NQ)
                                        for hi in range(2):
                                            h = 2 * hp + hi
                                            nc.tensor.matmul(
                                                cps[hi][:, psl],
                                                v_sb[:, pkc, h, 0:65],
                                                pu[:, hi],
                                                start=(pkc == 0),
                                                stop=(pkc == KC - 1))
                                    prev = (kc, qc, u2)
                            pkc, pqc, pu = prev
                            psl = slice(pqc * NQ, (pqc + 1) * NQ)
                            for hi in range(2):
                                h = 2 * hp + hi
                                nc.tensor.matmul(cps[hi][:, psl],
                                                 v_sb[:, pkc, h, 0:65],
                                                 pu[:, hi],
                                                 start=False, stop=True)
                            # finalize: evacuate ctx PSUM -> SBUF (fp16), then
                            # reciprocal of the denominator row via a [32,32]
                            # DRAM-roundtrip reshape, normalize on VectorE.
                            cus = []
                            for hi in range(2):
                                cu = pB.tile([65, NQH], F16, tag=f"cu{hi}")
                                nc.vector.tensor_copy(cu[:], cps[hi][:])
                                cus.append(cu)
                            rscrs, rsqs, rrecs, rscr2s, rbs = [], [], [], [], []
                            for hi in range(2):
                                rscr = dpool.tile([NQH], F16)
                                nc.gpsimd.dma_start(rscr[None, :],
                                                    cus[hi][64:65, :])
                                rscrs.append(rscr)
                            for hi in range(2):
                                rsq = pB.tile([32, NQH // 32], F16, tag=f"rsq{hi}")
                                nc.gpsimd.dma_start(
                                    rsq[:], rscrs[hi].rearrange("(a b) -> a b", a=32))
                                rsqs.append(rsq)
                            for hi in range(2):
                                rrec = pB.tile([32, NQH // 32], F16, tag=f"rrec{hi}")
                                with nc.allow_low_precision(
                                        reason="softmax denom fits fp16"):
                                    nc.vector.reciprocal(rrec[:], rsqs[hi][:])
                                rrecs.append(rrec)
                            for hi in range(2):
                                rscr2 = dpool.tile([NQH], F16)
                                nc.gpsimd.dma_start(
                                    rscr2.rearrange("(a b) -> a b", a=32), rrecs[hi][:])
                                rscr2s.append(rscr2)
                            for hi in range(2):
                                rb = pB.tile([64, NQH], F16, tag=f"rb{hi}")
                                nc.gpsimd.dma_start(rb[:],
                                                    rscr2s[hi].partition_broadcast(64))
                                rbs.append(rb)
                            for hi in range(2):
                                h = 2 * hp + hi
                                base = 64 * (h % 2)
                                ft = h // 2
                                nc.vector.tensor_mul(
                                    ctxT[base:base + 64, ft, qoff:qoff + NQH],
                                    cus[hi][0:64, :], rbs[hi][:])

                # ---------------- Phase C: output projection ----------------
                with (
                    tc.tile_pool(name="pC", bufs=4) as pC,
                    tc.tile_pool(name="psC", bufs=4, space="PSUM") as ppsC,
                ):
                    ci = 0
                    for tq in range(S // NT):
                        for ot in range(D // P):
                            tsl = slice(tq * NT, (tq + 1) * NT)
                            po = ppsC.tile([P, NT], F32, tag="po", name="po")
                            for fc in range(FT):
                                nc.tensor.matmul(po[:],
                                                 ow_sb[:, fc, ot * P:(ot + 1) * P],
                                                 ctxT[:, fc, tsl],
                                                 start=(fc == 0), stop=(fc == FT - 1))
                            o_sb = pC.tile([P, NT], F32, tag="oT")
                            if ci % 2 == 0:
                                nc.scalar.copy(o_sb[:], po[:])
                                nc.sync.dma_start(
                                    outT.ap()[ot * P:(ot + 1) * P, tsl], o_sb[:])
                            else:
                                nc.vector.tensor_copy(o_sb[:], po[:])
                                nc.scalar.dma_start(
                                    outT.ap()[ot * P:(ot + 1) * P, tsl], o_sb[:])
                            ci += 1

    nc.compile()
    return nc


def make_core_inputs(hidden_states, attention_bias, rope_cos, rope_sin, head_mask,
                     qkv_w, qkv_b, o_w, S=2048, D=1024):
    """Host-side sharding + layout preparation. Returns list of 8 input dicts."""
    f32 = np.float32
    f16 = np.float16
    mt = f16
    hidden_states = np.asarray(hidden_states, f32)
    attention_bias = np.asarray(attention_bias, f32)
    rope_cos = np.asarray(rope_cos, f32)
    rope_sin = np.asarray(rope_sin, f32)
    head_mask = np.asarray(head_mask, f32).reshape(-1)
    qkv_w = np.asarray(qkv_w, f32)
    qkv_b = np.asarray(qkv_b, f32)
    o_w = np.asarray(o_w, f32)

    B = hidden_states.shape[0]
    FPC = HPC * 64
    F = H * 64  # qkv feature dim (row-section size of qkv_w)

    def rot_rows(w):
        # rows indexed by f = hl*64 + d; rot(q)[d] = -q[d+32] (d<32) else q[d-32]
        w = w.reshape(HPC, 64, -1) if w.ndim == 2 else w.reshape(HPC, 64)
        lo, hi = w[:, 0:32], w[:, 32:64]
        out = np.concatenate([-hi, lo], axis=1)
        return out.reshape(HPC * 64, -1) if out.ndim == 3 else out.reshape(HPC * 64)

    cos_t = rope_cos[0, :, 0, :].T.astype(f32)     # [64, S]
    sin_t = rope_sin[0, :, 0, :].T.astype(f32)
    cosr = np.concatenate([cos_t, cos_t], axis=0)  # [128, S]
    sinr = np.concatenate([sin_t, sin_t], axis=0)

    in_maps = []
    for c in range(8):
        b, g = divmod(c, G)
        fs = slice(g * FPC, (g + 1) * FPC)
        wq = qkv_w[F * 0:F * 1][fs]
        wk = qkv_w[F * 1:F * 2][fs]
        wv = qkv_w[F * 2:F * 3][fs].copy()
        bq = qkv_b[F * 0:F * 1][fs]
        bk = qkv_b[F * 1:F * 2][fs]
        bvv = qkv_b[F * 2:F * 3][fs].copy()
        mask = head_mask[g * HPC:(g + 1) * HPC]
        wv *= np.repeat(mask, 64)[:, None]
        bvv *= np.repeat(mask, 64)
        w4 = np.concatenate([wq.T, wk.T], axis=1)  # [D, 2*FPC]
        b4 = np.concatenate([bq, bk])
        # lhsT for r = rot(u): out[p] = -u[p+32] (p%64<32) else u[p-32],
        # block-diagonal per 64-row head
        permM = np.zeros((128, 128), f32)
        for hb in (0, 64):
            for dd in range(32):
                permM[hb + dd + 32, hb + dd] = -1.0
                permM[hb + dd, hb + dd + 32] = 1.0
        bT = np.ascontiguousarray(attention_bias[b, 0].T)
        m = {
            "hT": np.ascontiguousarray(hidden_states[b].T).astype(mt),
            "w4": np.ascontiguousarray(w4).astype(mt),
            "b4": np.ascontiguousarray(b4),
            "permM": permM.astype(mt),
            "wvT": np.ascontiguousarray(wv.T).astype(mt),
            "bv": np.ascontiguousarray(bvv).astype(mt),
            "cosr": np.ascontiguousarray(cosr).astype(f16),
            "sinr": np.ascontiguousarray(sinr).astype(f16),
            "biasT": bT.astype(f16),
            "owT": np.ascontiguousarray(o_w[:, g * FPC:(g + 1) * FPC].T).astype(mt),
        }
        in_maps.append(m)
    return in_maps


def kernel(hidden_states, attention_bias, rope_cos, rope_sin, head_mask,
           qkv_w, qkv_b, o_w, o_b, **_unused):
    from concourse.bass_utils import run_bass_kernel_spmd

    B, S, D = hidden_states.shape
    if "nc" not in _CACHE:
        _CACHE["nc"] = build_nc(S=S, D=D)
    nc = _CACHE["nc"]

    in_maps = make_core_inputs(hidden_states, attention_bias, rope_cos, rope_sin,
                               head_mask, qkv_w, qkv_b, o_w, S=S, D=D)
    res = run_bass_kernel_spmd(nc, in_maps, list(range(8)))
    _CACHE["last_results"] = res

    o_b = np.asarray(o_b, np.float32)
    out = np.empty((B, S, D), np.float32)
    for b in range(B):
        acc = res.results[2 * b]["outT"].T + res.results[2 * b + 1]["outT"].T
        out[b] = acc + o_b[None, :]
    return out


# revision 33
# speedup vs baseline: 1.0237x; 1.0237x over previous
"""Trainium2 Bass kernel for nn_Attention_8143257993917.

Multi-head attention (packed QKV + RoPE + additive bias + softmax + head_mask
+ o_proj), B=4, S=2048, D=1024, H=16 heads, fp32 I/O.

Sharding: 8 cores = 4 batches x 2 head-groups (tensor-parallel over heads).
Core c handles batch b = c // 2 and heads g*8..g*8+8 with g = c % 2.
Each core computes a partial output (its heads' contribution through o_proj);
the host sums the two partials per batch and adds o_b.

Device-side design (per core), v2:
- Transposed feature-major layouts throughout (no on-chip transposes):
    Q_T, K_T: [f, t]; RoPE via second projection with host-prerotated weights.
    V: [t, f] natural, ones-column appended -> PV matmul also emits softmax
    denominators (row 64 of ctx psum).
- Phase B (attention) per (qh-half, head-pair):
    scores for both heads of a pair land in ONE [128, 2x512] PSUM tile
    (kc, qc)-granular, double-buffered (4 banks) so scores of step s+1 fill
    while exp of step s drains.  attention bias is added IN PSUM via an
    identity matmul (fp16 identity stationary, raw fp16 bias moving) --
    keeps the PE ~95% busy (HAM clock-gate stays warm) and frees VectorE.
    One ScalarE exp per step covers both heads ([128, 1024], PSUM->SBUF,
    fp16 out, softmax shift -12).  PV lags one step; ctx accumulates in the
    other 4 PSUM banks.  Denominator reciprocal via [32,32] DRAM-roundtrip
    reshape (fp16), normalize on VectorE in fp16.
- bias tile for the current q-half stays SBUF-resident (one 4MB load per
  half instead of per head-pair).
- Matmul dtype fp16 (same PE rate as bf16, ~8x lower quantization error).
"""

import sys

sys.path.insert(0, "/opt/trn_rl_repo")

import numpy as np

_CACHE = {}

H = 16
HPC = 8  # heads per core
G = 2  # head groups


def build_nc(S=2048, D=1024):
    """Build + compile the per-core Bass program (same program on all cores)."""
    import concourse.bass as bass
    from concourse import bacc
    import concourse.mybir as mybir
    import concourse.tile as tile
    from concourse.masks import make_identity
    from concourse.tile_rust import add_dep_helper

    F32 = mybir.dt.float32
    F16 = mybir.dt.float16
    MT = F16
    AF = mybir.ActivationFunctionType

    P = 128
    DC = D // P          # d chunks (contraction for projections)
    KC = S // P          # k chunks (scores contraction)
    NQH = S // 2         # q-half size
    NQ = 512             # scores/PV q-chunk (one psum bank)
    FPC = HPC * 64       # features per core (= 512)
    FT = FPC // P        # f-tiles per tensor (= 4)
    NT = 512             # phase C t-chunk
    NTA = 512            # phase A t-chunk

    nc = bacc.Bacc("TRN2", target_bir_lowering=False, debug=False, num_devices=8)

    hT = nc.dram_tensor("hT", [D, S], MT, kind="ExternalInput")
    w4 = nc.dram_tensor("w4", [D, 2 * FPC], MT, kind="ExternalInput")
    b4 = nc.dram_tensor("b4", [2 * FPC], F32, kind="ExternalInput")
    permM = nc.dram_tensor("permM", [P, P], MT, kind="ExternalInput")
    wvT = nc.dram_tensor("wvT", [D, FPC], MT, kind="ExternalInput")
    bv = nc.dram_tensor("bv", [FPC], MT, kind="ExternalInput")
    cosr = nc.dram_tensor("cosr", [P, S], F16, kind="ExternalInput")
    sinr = nc.dram_tensor("sinr", [P, S], F16, kind="ExternalInput")
    biasT = nc.dram_tensor("biasT", [S, S], F16, kind="ExternalInput")
    owT = nc.dram_tensor("owT", [FPC, D], MT, kind="ExternalInput")
    outT = nc.dram_tensor("outT", [D, S], F32, kind="ExternalOutput")

    hT_r = hT.ap().rearrange("(o p) t -> p o t", p=P)
    w4_r = w4.ap().rearrange("(o p) f -> p o f", p=P)
    wv_r = wvT.ap().rearrange("(o p) f -> p o f", p=P)
    ow_r = owT.ap().rearrange("(o p) f -> p o f", p=P)
    b4_r = b4.ap().rearrange("(o p) -> p o", p=P)
    bias_r = biasT.ap().rearrange("(kc p) q -> p kc q", p=P)

    with tile.TileContext(nc) as tc:
        with (
            tc.tile_pool(name="cst", bufs=1) as cst,
            tc.tile_pool(name="pAB", bufs=1) as pAB,
            tc.tile_pool(name="dram", bufs=4, space="DRAM") as dpool,
        ):
            identF = cst.tile([P, P], MT)
            make_identity(nc, identF)
            ones1 = cst.tile([1, P], MT)
            nc.vector.memset(ones1[:], 1.0)
            b4_sb = cst.tile([P, 2 * FPC // P], F32)
            bv_sb = cst.tile([1, FPC], MT)
            eshift = cst.tile([P, 1], F32)
            nc.vector.memset(eshift[:], -12.0)
            perm_sb = cst.tile([P, P], MT)

            # persistent phase A->B products
            qk_sb = pAB.tile([P, 2 * FT, S], MT)          # slots: Q ft 0..FT-1, K ft FT..2FT-1
            v_sb = pAB.tile([P, KC, HPC, 66], MT)          # col 64 = ones
            bias_sb = pAB.tile([P, 2, KC, NQH], F16)       # both q-halves
            # big bias loads ride the Activation HWDGE queue so they never
            # block the startup-critical loads on the sync/gpsimd queues
            nc.scalar.dma_start(bias_sb[:, 0], bias_r[:, :, 0:NQH])
            nc.scalar.dma_start(bias_sb[:, 1], bias_r[:, :, NQH:S])

            nc.vector.memset(v_sb[:, :, :, 64:65], 1.0)

            # ---------------- Phase A: projections + rope ----------------
            with (
                tc.tile_pool(name="pA", bufs=1) as pA,
                tc.tile_pool(name="pAw", bufs=2) as pAw,
                tc.tile_pool(name="psA", bufs=2, space="PSUM") as ppsA,
            ):
                first_wa = {}
                for half in range(2):
                    tsl = slice(half * NQH, (half + 1) * NQH)
                    if half == 0:
                        # hoist the first weight tile to the very front of
                        # the sync queue so the first matmul can start as
                        # soon as the first hidden-state chunk lands
                        wa0 = pAw.tile([P, DC, P], MT, tag="wA")
                        nc.sync.dma_start(wa0[:], w4_r[:, :, 0:P])
                        first_wa[(0, 0)] = wa0
                    cos_sb = pA.tile([P, NQH], F16, tag="cos", bufs=2)
                    nc.gpsimd.dma_start(cos_sb[:], cosr.ap()[:, tsl])
                    sin_sb = pA.tile([P, NQH], F16, tag="sin", bufs=2)
                    nc.gpsimd.dma_start(sin_sb[:], sinr.ap()[:, tsl])
                    if half == 0:
                        nc.gpsimd.dma_start(b4_sb[:], b4_r)
                        nc.gpsimd.dma_start(bv_sb[:], bv.ap()[None, :])
                        nc.gpsimd.dma_start(perm_sb[:], permM.ap())
                    h_sb = pA.tile([P, DC, NQH], MT, tag="hT", bufs=2)
                    for dc in range(DC):
                        nc.sync.dma_start(h_sb[:, dc], hT_r[:, dc, tsl])

                    # Q/K projections; RoPE rotate_half via a permutation
                    # matmul on the biased projection (rot(q + bq) includes
                    # the rotated bias automatically)
                    for qk in range(2):            # 0 = Q, 1 = K
                        for ft in range(FT):
                            fcol = qk * FPC + ft * P
                            wa = first_wa.pop((qk, ft), None) if half == 0 else None
                            if wa is None:
                                wa = pAw.tile([P, DC, P], MT, tag="wA")
                                nc.sync.dma_start(wa[:], w4_r[:, :, fcol:fcol + P])
                            bcol = qk * FT + ft
                            for tq in range(NQH // NTA):
                                qsl = slice(tq * NTA, (tq + 1) * NTA)
                                pa = ppsA.tile([P, NTA], F32, tag="pa", name="pa")
                                for dc in range(DC):
                                    nc.tensor.matmul(pa[:], wa[:, dc], h_sb[:, dc, qsl],
                                                     start=(dc == 0), stop=(dc == DC - 1))
                                ub = pAw.tile([P, NTA], F16, tag="ub")
                                nc.scalar.activation(ub[:], pa[:], AF.Identity,
                                                     bias=b4_sb[:, bcol:bcol + 1])
                                pr = ppsA.tile([P, NTA], F32, tag="pr", name="pr")
                                nc.tensor.matmul(pr[:], perm_sb[:], ub[:],
                                                 start=True, stop=True)
                                tca = pAw.tile([P, NTA], F16, tag="tca")
                                nc.vector.tensor_mul(tca[:], ub[:], cos_sb[:, qsl])
                                tcb = pAw.tile([P, NTA], F16, tag="tcb")
                                nc.vector.tensor_mul(tcb[:], pr[:], sin_sb[:, qsl])
                                dst = qk_sb[:, qk * FT + ft, half * NQH + tq * NTA:
                                            half * NQH + (tq + 1) * NTA]
                                nc.vector.tensor_add(dst, tca[:], tcb[:])

                    # big attention-bias load for this half rides the
                    # Activation HWDGE queue, emitted only now so its HBM
                    # traffic does not starve the startup-critical loads
                    nc.scalar.dma_start(bias_sb[:, half], bias_r[:, :, tsl])

                    # V for this half: t-tiles within half
                    wvs = pA.tile([P, DC, FPC], MT, tag="wV")
                    nc.gpsimd.dma_start(wvs[:], wv_r)
                    for tt in range(NQH // P):
                        gt = half * (NQH // P) + tt            # global t-tile = k-chunk
                        pv = ppsA.tile([P, FPC], F32, tag="pv", name="pv")
                        for dc in range(DC):
                            nc.tensor.matmul(pv[:], h_sb[:, dc, tt * P:(tt + 1) * P],
                                             wvs[:, dc], start=(dc == 0), stop=False)
                        nc.tensor.matmul(pv[:], ones1[:], bv_sb[:], start=False, stop=True)
                        nc.vector.tensor_copy(v_sb[:, gt, :, 0:64], pv[:])

            with tc.tile_pool(name="pBC", bufs=1) as pBC:
                ctxT = pBC.tile([P, FT, S], MT)            # normalized ctx, f-major
                ow_sb = pBC.tile([P, FT, D], MT)
                nc.sync.dma_start(ow_sb[:], ow_r)

                # ---------------- Phase B: attention ----------------
                with (
                    tc.tile_pool(name="pB", bufs=2) as pB,
                    tc.tile_pool(name="psB", bufs=1, space="PSUM") as ppsB,
                ):
                    for qh in range(2):
                        qoff = qh * NQH
                        for hp in range(HPC // 2):
                            cps = []
                            for i in range(2):
                                ct = ppsB.tile([P, NQH], F32, tag=f"ctx{i}",
                                               name=f"ctx{i}")
                                cps.append(ct[:65, :])
                            prev = None
                            for kc in range(KC):
                                for qc in range(2):
                                    csl = slice(qc * NQ, (qc + 1) * NQ)
                                    psS = ppsB.tile([P, 2, NQ], F32, tag="ps",
                                                    name="psS", bufs=2)
                                    prev_mm = None
                                    for hi in range(2):
                                        h = 2 * hp + hi
                                        base = 64 * (h % 2)
                                        ft = h // 2
                                        ksl = qk_sb[base:base + 64, FT + ft,
                                                    kc * P:(kc + 1) * P]
                                        qsl = qk_sb[base:base + 64, ft,
                                                    qoff + qc * NQ:
                                                    qoff + (qc + 1) * NQ]
                                        mm = nc.tensor.matmul(psS[:, hi], ksl,
                                                              qsl, start=True,
                                                              stop=False)
                                        if prev_mm is not None:
                                            add_dep_helper(
                                                mm.ins, prev_mm.ins, sync=False,
                                                reason="scores row-group pairing")
                                        prev_mm = mm
                                    for hi in range(2):
                                        nc.tensor.matmul(psS[:, hi], identF[:],
                                                         bias_sb[:, qh, kc, csl],
                                                         start=False, stop=True)
                                    u2 = pB.tile([P, 2, NQ], F16, tag="u2", bufs=3)
                                    nc.scalar.activation(u2[:], psS[:], AF.Exp,
                                                         bias=eshift[:])
                                    # software-pipeline: PV lags one step so the
                                    # PE never waits on this step's exp
                                    if prev is not None:
                                        pkc, pqc, pu = prev
                                        psl = slice(pqc * NQ, (pqc + 1) * NQ)
                                        for hi in range(2):
                                            h = 2 * hp + hi
                                            nc.tensor.matmul(
                                                cps[hi][:, psl],
                                                v_sb[:, pkc, h, 0:65],
                                                pu[:, hi],
                                                start=(pkc == 0),
                                                stop=(pkc == KC - 1))
                                    prev = (kc, qc, u2)
                            # finalize: evacuate ctx PSUM -> SBUF (fp16), then
                            # reciprocal of the denominator row via a [32,32]
                            # DRAM-roundtrip reshape, normalize on VectorE.
                            # qc0's ctx columns are already complete here, so
                            # their evacuation overlaps the trailing PV.
                            cus = []
                            for hi in range(2):
                                cu = pB.tile([65, NQH], F16, tag=f"cu{hi}")
                                nc.vector.tensor_copy(cu[:, 0:NQ], cps[hi][:, 0:NQ])
                                cus.append(cu)
                            pkc, pqc, pu = prev
                            psl = slice(pqc * NQ, (pqc + 1) * NQ)
                            for hi in range(2):
                                h = 2 * hp + hi
                                nc.tensor.matmul(cps[hi][:, psl],
                                                 v_sb[:, pkc, h, 0:65],
                                                 pu[:, hi],
                                                 start=False, stop=True)
                            for hi in range(2):
                                nc.vector.tensor_copy(cus[hi][:, NQ:NQH],
                                                      cps[hi][:, NQ:NQH])
                            rscrs, rsqs, rrecs, rscr2s, rbs = [], [], [], [], []
                            for hi in range(2):
                                rscr = dpool.tile([NQH], F16)
                                nc.gpsimd.dma_start(rscr[None, :],
                                                    cus[hi][64:65, :])
                                rscrs.append(rscr)
                            for hi in range(2):
                                rsq = pB.tile([32, NQH // 32], F16, tag=f"rsq{hi}")
                                nc.gpsimd.dma_start(
                                    rsq[:], rscrs[hi].rearrange("(a b) -> a b", a=32))
                                rsqs.append(rsq)
                            for hi in range(2):
                                rrec = pB.tile([32, NQH // 32], F16, tag=f"rrec{hi}")
                                with nc.allow_low_precision(
                                        reason="softmax denom fits fp16"):
                                    nc.vector.reciprocal(rrec[:], rsqs[hi][:])
                                rrecs.append(rrec)
                            for hi in range(2):
                                rscr2 = dpool.tile([NQH], F16)
                                nc.gpsimd.dma_start(
                                    rscr2.rearrange("(a b) -> a b", a=32), rrecs[hi][:])
                                rscr2s.append(rscr2)
                            for hi in range(2):
                                rb = pB.tile([64, NQH], F16, tag=f"rb{hi}")
                                nc.gpsimd.dma_start(rb[:],
                                                    rscr2s[hi].partition_broadcast(64))
                                rbs.append(rb)
                            for hi in range(2):
                                h = 2 * hp + hi
                                base = 64 * (h % 2)
                                ft = h // 2
                                nc.vector.tensor_mul(
                                    ctxT[base:base + 64, ft, qoff:qoff + NQH],
                                    cus[hi][0:64, :], rbs[hi][:])

                # ---------------- Phase C: output projection ----------------
                with (
                    tc.tile_pool(name="pC", bufs=4) as pC,
                    tc.tile_pool(name="psC", bufs=4, space="PSUM") as ppsC,
                ):
                    ci = 0
                    for tq in range(S // NT):
                        for ot in range(D // P):
                            tsl = slice(tq * NT, (tq + 1) * NT)
                            po = ppsC.tile([P, NT], F32, tag="po", name="po")
                            for fc in range(FT):
                                nc.tensor.matmul(po[:],
                                                 ow_sb[:, fc, ot * P:(ot + 1) * P],
                                                 ctxT[:, fc, tsl],
                                                 start=(fc == 0), stop=(fc == FT - 1))
                            o_sb = pC.tile([P, NT], F32, tag="oT")
                            if ci % 2 == 0:
                                nc.scalar.copy(o_sb[:], po[:])
                                nc.sync.dma_start(
                                    outT.ap()[ot * P:(ot + 1) * P, tsl], o_sb[:])
                            else:
                                nc.vector.tensor_copy(o_sb[:], po[:])
                                nc.scalar.dma_start(
                                    outT.ap()[ot * P:(ot + 1) * P, tsl], o_sb[:])
                            ci += 1

    nc.compile()
    return nc


def make_core_inputs(hidden_states, attention_bias, rope_cos, rope_sin, head_mask,
                     qkv_w, qkv_b, o_w, S=2048, D=1024):
    """Host-side sharding + layout preparation. Returns list of 8 input dicts."""
    f32 = np.float32
    f16 = np.float16
    mt = f16
    hidden_states = np.asarray(hidden_states, f32)
    attention_bias = np.asarray(attention_bias, f32)
    rope_cos = np.asarray(rope_cos, f32)
    rope_sin = np.asarray(rope_sin, f32)
    head_mask = np.asarray(head_mask, f32).reshape(-1)
    qkv_w = np.asarray(qkv_w, f32)
    qkv_b = np.asarray(qkv_b, f32)
    o_w = np.asarray(o_w, f32)

    B = hidden_states.shape[0]
    FPC = HPC * 64
    F = H * 64  # qkv feature dim (row-section size of qkv_w)

    def rot_rows(w):
        # rows indexed by f = hl*64 + d; rot(q)[d] = -q[d+32] (d<32) else q[d-32]
        w = w.reshape(HPC, 64, -1) if w.ndim == 2 else w.reshape(HPC, 64)
        lo, hi = w[:, 0:32], w[:, 32:64]
        out = np.concatenate([-hi, lo], axis=1)
        return out.reshape(HPC * 64, -1) if out.ndim == 3 else out.reshape(HPC * 64)

    cos_t = rope_cos[0, :, 0, :].T.astype(f32)     # [64, S]
    sin_t = rope_sin[0, :, 0, :].T.astype(f32)
    cosr = np.concatenate([cos_t, cos_t], axis=0)  # [128, S]
    sinr = np.concatenate([sin_t, sin_t], axis=0)

    in_maps = []
    for c in range(8):
        b, g = divmod(c, G)
        fs = slice(g * FPC, (g + 1) * FPC)
        wq = qkv_w[F * 0:F * 1][fs]
        wk = qkv_w[F * 1:F * 2][fs]
        wv = qkv_w[F * 2:F * 3][fs].copy()
        bq = qkv_b[F * 0:F * 1][fs]
        bk = qkv_b[F * 1:F * 2][fs]
        bvv = qkv_b[F * 2:F * 3][fs].copy()
        mask = head_mask[g * HPC:(g + 1) * HPC]
        wv *= np.repeat(mask, 64)[:, None]
        bvv *= np.repeat(mask, 64)
        w4 = np.concatenate([wq.T, wk.T], axis=1)  # [D, 2*FPC]
        b4 = np.concatenate([bq, bk])
        # lhsT for r = rot(u): out[p] = -u[p+32] (p%64<32) else u[p-32],
        # block-diagonal per 64-row head
        permM = np.zeros((128, 128), f32)
        for hb in (0, 64):
            for dd in range(32):
                permM[hb + dd + 32, hb + dd] = -1.0
                permM[hb + dd, hb + dd + 32] = 1.0
        bT = np.ascontiguousarray(attention_bias[b, 0].T)
        m = {
            "hT": np.ascontiguousarray(hidden_states[b].T).astype(mt),
            "w4": np.ascontiguousarray(w4).astype(mt),
            "b4": np.ascontiguousarray(b4),
            "permM": permM.astype(mt),
            "wvT": np.ascontiguousarray(wv.T).astype(mt),
            "bv": np.ascontiguousarray(bvv).astype(mt),
            "cosr": np.ascontiguousarray(cosr).astype(f16),
            "sinr": np.ascontiguousarray(sinr).astype(f16),
            "biasT": bT.astype(f16),
            "owT": np.ascontiguousarray(o_w[:, g * FPC:(g + 1) * FPC].T).astype(mt),
        }
        in_maps.append(m)
    return in_maps


def kernel(hidden_states, attention_bias, rope_cos, rope_sin, head_mask,
           qkv_w, qkv_b, o_w, o_b, **_unused):
    from concourse.bass_utils import run_bass_kernel_spmd

    B, S, D = hidden_states.shape
    if "nc" not in _CACHE:
        _CACHE["nc"] = build_nc(S=S, D=D)
    nc = _CACHE["nc"]

    in_maps = make_core_inputs(hidden_states, attention_bias, rope_cos, rope_sin,
                               head_mask, qkv_w, qkv_b, o_w, S=S, D=D)
    res = run_bass_kernel_spmd(nc, in_maps, list(range(8)))
    _CACHE["last_results"] = res

    o_b = np.asarray(o_b, np.float32)
    out = np.empty((B, S, D), np.float32)
    for b in range(B):
        acc = res.results[2 * b]["outT"].T + res.results[2 * b + 1]["outT"].T
        out[b] = acc + o_b[None, :]
    return out


# revision 36
# speedup vs baseline: 1.1269x; 1.1008x over previous
"""Trainium2 Bass kernel for nn_Attention_8143257993917.

Multi-head attention (packed QKV + RoPE + additive bias + softmax + head_mask
+ o_proj), B=4, S=2048, D=1024, H=16 heads, fp32 I/O.

Sharding: 8 cores = 4 batches x 2 head-groups (tensor-parallel over heads).
Core c handles batch b = c // 2 and heads g*8..g*8+8 with g = c % 2.
Each core computes a partial output (its heads' contribution through o_proj);
the host sums the two partials per batch and adds o_b.

Device-side design (per core), v2:
- Transposed feature-major layouts throughout (no on-chip transposes):
    Q_T, K_T: [f, t]; RoPE via second projection with host-prerotated weights.
    V: [t, f] natural, ones-column appended -> PV matmul also emits softmax
    denominators (row 64 of ctx psum).
- Phase B (attention) per (qh-half, head-pair):
    scores for both heads of a pair land in ONE [128, 2x512] PSUM tile
    (kc, qc)-granular, double-buffered (4 banks) so scores of step s+1 fill
    while exp of step s drains.  attention bias is added IN PSUM via an
    identity matmul (fp16 identity stationary, raw fp16 bias moving) --
    keeps the PE ~95% busy (HAM clock-gate stays warm) and frees VectorE.
    One ScalarE exp per step covers both heads ([128, 1024], PSUM->SBUF,
    fp16 out, softmax shift -12).  PV lags one step; ctx accumulates in the
    other 4 PSUM banks.  Denominator reciprocal via [32,32] DRAM-roundtrip
    reshape (fp16), normalize on VectorE in fp16.
- bias tile for the current q-half stays SBUF-resident (one 4MB load per
  half instead of per head-pair).
- Matmul dtype fp16 (same PE rate as bf16, ~8x lower quantization error).
"""

import sys

sys.path.insert(0, "/opt/trn_rl_repo")

import numpy as np

_CACHE = {}

H = 16
HPC = 8  # heads per core
G = 2  # head groups


def build_nc(S=2048, D=1024):
    """Build + compile the per-core Bass program (same program on all cores)."""
    import concourse.bass as bass
    from concourse import bacc
    import concourse.mybir as mybir
    import concourse.tile as tile
    from concourse.masks import make_identity
    from concourse.tile_rust import add_dep_helper

    F32 = mybir.dt.float32
    F16 = mybir.dt.float16
    MT = F16
    AF = mybir.ActivationFunctionType

    P = 128
    DC = D // P          # d chunks (contraction for projections)
    KC = S // P          # k chunks (scores contraction)
    NQH = S // 2         # q-half size
    NQ = 512             # scores/PV q-chunk (one psum bank)
    FPC = HPC * 64       # features per core (= 512)
    FT = FPC // P        # f-tiles per tensor (= 4)
    NT = 512             # phase C t-chunk
    NTA = 512            # phase A t-chunk

    nc = bacc.Bacc("TRN2", target_bir_lowering=False, debug=False, num_devices=8)

    hT = nc.dram_tensor("hT", [D, S], MT, kind="ExternalInput")
    w4 = nc.dram_tensor("w4", [D, 2 * FPC], MT, kind="ExternalInput")
    b4 = nc.dram_tensor("b4", [2 * FPC], F32, kind="ExternalInput")
    permM = nc.dram_tensor("permM", [P, P], MT, kind="ExternalInput")
    wvT = nc.dram_tensor("wvT", [D, FPC], MT, kind="ExternalInput")
    bv = nc.dram_tensor("bv", [FPC], MT, kind="ExternalInput")
    cosr = nc.dram_tensor("cosr", [P, S], F16, kind="ExternalInput")
    sinr = nc.dram_tensor("sinr", [P, S], F16, kind="ExternalInput")
    biasT = nc.dram_tensor("biasT", [S, S], F16, kind="ExternalInput")
    owT = nc.dram_tensor("owT", [FPC, D], MT, kind="ExternalInput")
    outT = nc.dram_tensor("outT", [D, S], F32, kind="ExternalOutput")

    hT_r = hT.ap().rearrange("(o p) t -> p o t", p=P)
    w4_r = w4.ap().rearrange("(o p) f -> p o f", p=P)
    wv_r = wvT.ap().rearrange("(o p) f -> p o f", p=P)
    ow_r = owT.ap().rearrange("(o p) f -> p o f", p=P)
    b4_r = b4.ap().rearrange("(o p) -> p o", p=P)
    bias_r = biasT.ap().rearrange("(kc p) q -> p kc q", p=P)

    with tile.TileContext(nc) as tc:
        with (
            tc.tile_pool(name="cst", bufs=1) as cst,
            tc.tile_pool(name="pAB", bufs=1) as pAB,
            tc.tile_pool(name="dram", bufs=4, space="DRAM") as dpool,
        ):
            identF = cst.tile([P, P], MT)
            make_identity(nc, identF)
            ones1 = cst.tile([1, P], MT)
            nc.vector.memset(ones1[:], 1.0)
            b4_sb = cst.tile([P, 2 * FPC // P], F32)
            bv_sb = cst.tile([1, FPC], MT)
            eshift = cst.tile([P, 1], F32)
            nc.vector.memset(eshift[:], -12.0)
            perm_sb = cst.tile([P, P], MT)

            # persistent phase A->B products
            qk_sb = pAB.tile([P, 2 * FT, S], MT)          # slots: Q ft 0..FT-1, K ft FT..2FT-1
            v_sb = pAB.tile([P, KC, HPC, 66], MT)          # col 64 = ones
            bias_sb = pAB.tile([P, 2, KC, NQH], F16)       # both q-halves
            # big bias loads ride the Activation HWDGE queue so they never
            # block the startup-critical loads on the sync/gpsimd queues
            nc.scalar.dma_start(bias_sb[:, 0], bias_r[:, :, 0:NQH])
            nc.scalar.dma_start(bias_sb[:, 1], bias_r[:, :, NQH:S])

            nc.vector.memset(v_sb[:, :, :, 64:65], 1.0)

            # ---------------- Phase A: projections + rope ----------------
            with (
                tc.tile_pool(name="pA", bufs=1) as pA,
                tc.tile_pool(name="pAw", bufs=2) as pAw,
                tc.tile_pool(name="psA", bufs=2, space="PSUM") as ppsA,
            ):
                first_wa = {}
                for half in range(2):
                    tsl = slice(half * NQH, (half + 1) * NQH)
                    if half == 0:
                        # hoist the first weight tile to the very front of
                        # the sync queue so the first matmul can start as
                        # soon as the first hidden-state chunk lands
                        wa0 = pAw.tile([P, DC, P], MT, tag="wA")
                        nc.sync.dma_start(wa0[:], w4_r[:, :, 0:P])
                        first_wa[(0, 0)] = wa0
                    cos_sb = pA.tile([P, NQH], F16, tag="cos", bufs=2)
                    nc.gpsimd.dma_start(cos_sb[:], cosr.ap()[:, tsl])
                    sin_sb = pA.tile([P, NQH], F16, tag="sin", bufs=2)
                    nc.gpsimd.dma_start(sin_sb[:], sinr.ap()[:, tsl])
                    if half == 0:
                        nc.gpsimd.dma_start(b4_sb[:], b4_r)
                        nc.gpsimd.dma_start(bv_sb[:], bv.ap()[None, :])
                        nc.gpsimd.dma_start(perm_sb[:], permM.ap())
                    h_sb = pA.tile([P, DC, NQH], MT, tag="hT", bufs=2)
                    for dc in range(DC):
                        nc.sync.dma_start(h_sb[:, dc], hT_r[:, dc, tsl])

                    # Q/K projections; RoPE rotate_half via a permutation
                    # matmul on the biased projection (rot(q + bq) includes
                    # the rotated bias automatically)
                    for qk in range(2):            # 0 = Q, 1 = K
                        for ft in range(FT):
                            fcol = qk * FPC + ft * P
                            wa = first_wa.pop((qk, ft), None) if half == 0 else None
                            if wa is None:
                                wa = pAw.tile([P, DC, P], MT, tag="wA")
                                nc.sync.dma_start(wa[:], w4_r[:, :, fcol:fcol + P])
                            bcol = qk * FT + ft
                            for tq in range(NQH // NTA):
                                qsl = slice(tq * NTA, (tq + 1) * NTA)
                                pa = ppsA.tile([P, NTA], F32, tag="pa", name="pa")
                                for dc in range(DC):
                                    nc.tensor.matmul(pa[:], wa[:, dc], h_sb[:, dc, qsl],
                                                     start=(dc == 0), stop=(dc == DC - 1))
                                ub = pAw.tile([P, NTA], F16, tag="ub")
                                nc.scalar.activation(ub[:], pa[:], AF.Identity,
                                                     bias=b4_sb[:, bcol:bcol + 1])
                                pr = ppsA.tile([P, NTA], F32, tag="pr", name="pr")
                                nc.tensor.matmul(pr[:], perm_sb[:], ub[:],
                                                 start=True, stop=True)
                                tca = pAw.tile([P, NTA], F16, tag="tca")
                                nc.vector.tensor_mul(tca[:], ub[:], cos_sb[:, qsl])
                                tcb = pAw.tile([P, NTA], F16, tag="tcb")
                                nc.vector.tensor_mul(tcb[:], pr[:], sin_sb[:, qsl])
                                dst = qk_sb[:, qk * FT + ft, half * NQH + tq * NTA:
                                            half * NQH + (tq + 1) * NTA]
                                nc.vector.tensor_add(dst, tca[:], tcb[:])

                    # big attention-bias load for this half rides the
                    # Activation HWDGE queue, emitted only now so its HBM
                    # traffic does not starve the startup-critical loads
                    nc.scalar.dma_start(bias_sb[:, half], bias_r[:, :, tsl])

                    # V for this half: t-tiles within half
                    wvs = pA.tile([P, DC, FPC], MT, tag="wV")
                    nc.gpsimd.dma_start(wvs[:], wv_r)
                    for tt in range(NQH // P):
                        gt = half * (NQH // P) + tt            # global t-tile = k-chunk
                        pv = ppsA.tile([P, FPC], F32, tag="pv", name="pv")
                        for dc in range(DC):
                            nc.tensor.matmul(pv[:], h_sb[:, dc, tt * P:(tt + 1) * P],
                                             wvs[:, dc], start=(dc == 0), stop=False)
                        nc.tensor.matmul(pv[:], ones1[:], bv_sb[:], start=False, stop=True)
                        nc.vector.tensor_copy(v_sb[:, gt, :, 0:64], pv[:])

            with tc.tile_pool(name="pBC", bufs=1) as pBC:
                ctxT = pBC.tile([P, FT, S], MT)            # normalized ctx, f-major
                ow_sb = pBC.tile([P, FT, D], MT)
                nc.sync.dma_start(ow_sb[:], ow_r)

                # ---------------- Phase B: attention ----------------
                with (
                    tc.tile_pool(name="pB", bufs=2) as pB,
                    tc.tile_pool(name="psB", bufs=1, space="PSUM") as ppsB,
                ):
                    for qh in range(2):
                        qoff = qh * NQH
                        for hp in range(HPC // 2):
                            cps = []
                            for i in range(2):
                                ct = ppsB.tile([P, NQH], F32, tag=f"ctx{i}",
                                               name=f"ctx{i}")
                                cps.append(ct[:65, :])
                            prev = None
                            for kc in range(KC):
                                for qc in range(2):
                                    csl = slice(qc * NQ, (qc + 1) * NQ)
                                    psS = ppsB.tile([P, 2, NQ], F32, tag="ps",
                                                    name="psS", bufs=2)
                                    prev_mm = None
                                    for hi in range(2):
                                        h = 2 * hp + hi
                                        base = 64 * (h % 2)
                                        ft = h // 2
                                        ksl = qk_sb[base:base + 64, FT + ft,
                                                    kc * P:(kc + 1) * P]
                                        qsl = qk_sb[base:base + 64, ft,
                                                    qoff + qc * NQ:
                                                    qoff + (qc + 1) * NQ]
                                        mm = nc.tensor.matmul(psS[:, hi], ksl,
                                                              qsl, start=True,
                                                              stop=True)
                                        if prev_mm is not None:
                                            add_dep_helper(
                                                mm.ins, prev_mm.ins, sync=False,
                                                reason="scores row-group pairing")
                                        prev_mm = mm
                                    u2e = pB.tile([P, 2, NQ], F16, tag="u2e",
                                                  bufs=3)
                                    nc.scalar.activation(u2e[:], psS[:], AF.Exp,
                                                         bias=eshift[:])
                                    # exp(s+b) = exp(s)*exp(b): bias applied as
                                    # a VectorE multiply (bias_sb holds exp(b))
                                    u2 = pB.tile([P, 2, NQ], F16, tag="u2", bufs=3)
                                    for hi in range(2):
                                        nc.vector.tensor_mul(
                                            u2[:, hi], u2e[:, hi],
                                            bias_sb[:, qh, kc, csl])
                                    # software-pipeline: PV lags one step so the
                                    # PE never waits on this step's exp
                                    if prev is not None:
                                        pkc, pqc, pu = prev
                                        psl = slice(pqc * NQ, (pqc + 1) * NQ)
                                        for hi in range(2):
                                            h = 2 * hp + hi
                                            nc.tensor.matmul(
                                                cps[hi][:, psl],
                                                v_sb[:, pkc, h, 0:65],
                                                pu[:, hi],
                                                start=(pkc == 0),
                                                stop=(pkc == KC - 1))
                                    prev = (kc, qc, u2)
                            # finalize: evacuate ctx PSUM -> SBUF (fp16), then
                            # reciprocal of the denominator row via a [32,32]
                            # DRAM-roundtrip reshape, normalize on VectorE.
                            # qc0's ctx columns are already complete here, so
                            # their evacuation overlaps the trailing PV.
                            cus = []
                            for hi in range(2):
                                cu = pB.tile([65, NQH], F16, tag=f"cu{hi}")
                                nc.vector.tensor_copy(cu[:, 0:NQ], cps[hi][:, 0:NQ])
                                cus.append(cu)
                            pkc, pqc, pu = prev
                            psl = slice(pqc * NQ, (pqc + 1) * NQ)
                            for hi in range(2):
                                h = 2 * hp + hi
                                nc.tensor.matmul(cps[hi][:, psl],
                                                 v_sb[:, pkc, h, 0:65],
                                                 pu[:, hi],
                                                 start=False, stop=True)
                            for hi in range(2):
                                nc.vector.tensor_copy(cus[hi][:, NQ:NQH],
                                                      cps[hi][:, NQ:NQH])
                            rscrs, rsqs, rrecs, rscr2s, rbs = [], [], [], [], []
                            for hi in range(2):
                                rscr = dpool.tile([NQH], F16)
                                nc.gpsimd.dma_start(rscr[None, :],
                                                    cus[hi][64:65, :])
                                rscrs.append(rscr)
                            for hi in range(2):
                                rsq = pB.tile([32, NQH // 32], F16, tag=f"rsq{hi}")
                                nc.gpsimd.dma_start(
                                    rsq[:], rscrs[hi].rearrange("(a b) -> a b", a=32))
                                rsqs.append(rsq)
                            for hi in range(2):
                                rrec = pB.tile([32, NQH // 32], F16, tag=f"rrec{hi}")
                                with nc.allow_low_precision(
                                        reason="softmax denom fits fp16"):
                                    nc.vector.reciprocal(rrec[:], rsqs[hi][:])
                                rrecs.append(rrec)
                            for hi in range(2):
                                rscr2 = dpool.tile([NQH], F16)
                                nc.gpsimd.dma_start(
                                    rscr2.rearrange("(a b) -> a b", a=32), rrecs[hi][:])
                                rscr2s.append(rscr2)
                            for hi in range(2):
                                rb = pB.tile([64, NQH], F16, tag=f"rb{hi}")
                                nc.gpsimd.dma_start(rb[:],
                                                    rscr2s[hi].partition_broadcast(64))
                                rbs.append(rb)
                            for hi in range(2):
                                h = 2 * hp + hi
                                base = 64 * (h % 2)
                                ft = h // 2
                                nc.vector.tensor_mul(
                                    ctxT[base:base + 64, ft, qoff:qoff + NQH],
                                    cus[hi][0:64, :], rbs[hi][:])

                # ---------------- Phase C: output projection ----------------
                with (
                    tc.tile_pool(name="pC", bufs=4) as pC,
                    tc.tile_pool(name="psC", bufs=4, space="PSUM") as ppsC,
                ):
                    ci = 0
                    for tq in range(S // NT):
                        for ot in range(D // P):
                            tsl = slice(tq * NT, (tq + 1) * NT)
                            po = ppsC.tile([P, NT], F32, tag="po", name="po")
                            for fc in range(FT):
                                nc.tensor.matmul(po[:],
                                                 ow_sb[:, fc, ot * P:(ot + 1) * P],
                                                 ctxT[:, fc, tsl],
                                                 start=(fc == 0), stop=(fc == FT - 1))
                            o_sb = pC.tile([P, NT], F32, tag="oT")
                            if ci % 2 == 0:
                                nc.scalar.copy(o_sb[:], po[:])
                                nc.sync.dma_start(
                                    outT.ap()[ot * P:(ot + 1) * P, tsl], o_sb[:])
                            else:
                                nc.vector.tensor_copy(o_sb[:], po[:])
                                nc.scalar.dma_start(
                                    outT.ap()[ot * P:(ot + 1) * P, tsl], o_sb[:])
                            ci += 1

    nc.compile()
    return nc


def make_core_inputs(hidden_states, attention_bias, rope_cos, rope_sin, head_mask,
                     qkv_w, qkv_b, o_w, S=2048, D=1024):
    """Host-side sharding + layout preparation. Returns list of 8 input dicts."""
    f32 = np.float32
    f16 = np.float16
    mt = f16
    hidden_states = np.asarray(hidden_states, f32)
    attention_bias = np.asarray(attention_bias, f32)
    rope_cos = np.asarray(rope_cos, f32)
    rope_sin = np.asarray(rope_sin, f32)
    head_mask = np.asarray(head_mask, f32).reshape(-1)
    qkv_w = np.asarray(qkv_w, f32)
    qkv_b = np.asarray(qkv_b, f32)
    o_w = np.asarray(o_w, f32)

    B = hidden_states.shape[0]
    FPC = HPC * 64
    F = H * 64  # qkv feature dim (row-section size of qkv_w)

    def rot_rows(w):
        # rows indexed by f = hl*64 + d; rot(q)[d] = -q[d+32] (d<32) else q[d-32]
        w = w.reshape(HPC, 64, -1) if w.ndim == 2 else w.reshape(HPC, 64)
        lo, hi = w[:, 0:32], w[:, 32:64]
        out = np.concatenate([-hi, lo], axis=1)
        return out.reshape(HPC * 64, -1) if out.ndim == 3 else out.reshape(HPC * 64)

    cos_t = rope_cos[0, :, 0, :].T.astype(f32)     # [64, S]
    sin_t = rope_sin[0, :, 0, :].T.astype(f32)
    cosr = np.concatenate([cos_t, cos_t], axis=0)  # [128, S]
    sinr = np.concatenate([sin_t, sin_t], axis=0)

    in_maps = []
    for c in range(8):
        b, g = divmod(c, G)
        fs = slice(g * FPC, (g + 1) * FPC)
        wq = qkv_w[F * 0:F * 1][fs]
        wk = qkv_w[F * 1:F * 2][fs]
        wv = qkv_w[F * 2:F * 3][fs].copy()
        bq = qkv_b[F * 0:F * 1][fs]
        bk = qkv_b[F * 1:F * 2][fs]
        bvv = qkv_b[F * 2:F * 3][fs].copy()
        mask = head_mask[g * HPC:(g + 1) * HPC]
        wv *= np.repeat(mask, 64)[:, None]
        bvv *= np.repeat(mask, 64)
        w4 = np.concatenate([wq.T, wk.T], axis=1)  # [D, 2*FPC]
        b4 = np.concatenate([bq, bk])
        # lhsT for r = rot(u): out[p] = -u[p+32] (p%64<32) else u[p-32],
        # block-diagonal per 64-row head
        permM = np.zeros((128, 128), f32)
        for hb in (0, 64):
            for dd in range(32):
                permM[hb + dd + 32, hb + dd] = -1.0
                permM[hb + dd, hb + dd + 32] = 1.0
        bT = np.ascontiguousarray(attention_bias[b, 0].T)
        m = {
            "hT": np.ascontiguousarray(hidden_states[b].T).astype(mt),
            "w4": np.ascontiguousarray(w4).astype(mt),
            "b4": np.ascontiguousarray(b4),
            "permM": permM.astype(mt),
            "wvT": np.ascontiguousarray(wv.T).astype(mt),
            "bv": np.ascontiguousarray(bvv).astype(mt),
            "cosr": np.ascontiguousarray(cosr).astype(f16),
            "sinr": np.ascontiguousarray(sinr).astype(f16),
            "biasT": np.exp(bT).astype(f16),
            "owT": np.ascontiguousarray(o_w[:, g * FPC:(g + 1) * FPC].T).astype(mt),
        }
        in_maps.append(m)
    return in_maps


def kernel(hidden_states, attention_bias, rope_cos, rope_sin, head_mask,
           qkv_w, qkv_b, o_w, o_b, **_unused):
    from concourse.bass_utils import run_bass_kernel_spmd

    B, S, D = hidden_states.shape
    if "nc" not in _CACHE:
        _CACHE["nc"] = build_nc(S=S, D=D)
    nc = _CACHE["nc"]

    in_maps = make_core_inputs(hidden_states, attention_bias, rope_cos, rope_sin,
                               head_mask, qkv_w, qkv_b, o_w, S=S, D=D)
    res = run_bass_kernel_spmd(nc, in_maps, list(range(8)))
    _CACHE["last_results"] = res

    o_b = np.asarray(o_b, np.float32)
    out = np.empty((B, S, D), np.float32)
    for b in range(B):
        acc = res.results[2 * b]["outT"].T + res.results[2 * b + 1]["outT"].T
        out[b] = acc + o_b[None, :]
    return out


# revision 38
# speedup vs baseline: 1.1511x; 1.0215x over previous
"""Trainium2 Bass kernel for nn_Attention_8143257993917.

Multi-head attention (packed QKV + RoPE + additive bias + softmax + head_mask
+ o_proj), B=4, S=2048, D=1024, H=16 heads, fp32 I/O.

Sharding: 8 cores = 4 batches x 2 head-groups (tensor-parallel over heads).
Core c handles batch b = c // 2 and heads g*8..g*8+8 with g = c % 2.
Each core computes a partial output (its heads' contribution through o_proj);
the host sums the two partials per batch and adds o_b.

Device-side design (per core), v2:
- Transposed feature-major layouts throughout (no on-chip transposes):
    Q_T, K_T: [f, t]; RoPE via second projection with host-prerotated weights.
    V: [t, f] natural, ones-column appended -> PV matmul also emits softmax
    denominators (row 64 of ctx psum).
- Phase B (attention) per (qh-half, head-pair):
    scores for both heads of a pair land in ONE [128, 2x512] PSUM tile
    (kc, qc)-granular, double-buffered (4 banks) so scores of step s+1 fill
    while exp of step s drains.  attention bias is added IN PSUM via an
    identity matmul (fp16 identity stationary, raw fp16 bias moving) --
    keeps the PE ~95% busy (HAM clock-gate stays warm) and frees VectorE.
    One ScalarE exp per step covers both heads ([128, 1024], PSUM->SBUF,
    fp16 out, softmax shift -12).  PV lags one step; ctx accumulates in the
    other 4 PSUM banks.  Denominator reciprocal via [32,32] DRAM-roundtrip
    reshape (fp16), normalize on VectorE in fp16.
- bias tile for the current q-half stays SBUF-resident (one 4MB load per
  half instead of per head-pair).
- Matmul dtype fp16 (same PE rate as bf16, ~8x lower quantization error).
"""

import sys

sys.path.insert(0, "/opt/trn_rl_repo")

import numpy as np

_CACHE = {}

H = 16
HPC = 8  # heads per core
G = 2  # head groups


def build_nc(S=2048, D=1024):
    """Build + compile the per-core Bass program (same program on all cores)."""
    import concourse.bass as bass
    from concourse import bacc
    import concourse.mybir as mybir
    import concourse.tile as tile
    from concourse.masks import make_identity
    from concourse.tile_rust import add_dep_helper

    F32 = mybir.dt.float32
    F16 = mybir.dt.float16
    MT = F16
    AF = mybir.ActivationFunctionType

    P = 128
    DC = D // P          # d chunks (contraction for projections)
    KC = S // P          # k chunks (scores contraction)
    NQH = S // 2         # q-half size
    NQ = 512             # scores/PV q-chunk (one psum bank)
    FPC = HPC * 64       # features per core (= 512)
    FT = FPC // P        # f-tiles per tensor (= 4)
    NT = 512             # phase C t-chunk
    NTA = 512            # phase A t-chunk

    nc = bacc.Bacc("TRN2", target_bir_lowering=False, debug=False, num_devices=8)

    hT = nc.dram_tensor("hT", [D, S], MT, kind="ExternalInput")
    w4 = nc.dram_tensor("w4", [D, 2 * FPC], MT, kind="ExternalInput")
    b4 = nc.dram_tensor("b4", [2 * FPC], F32, kind="ExternalInput")
    permM = nc.dram_tensor("permM", [P, P], MT, kind="ExternalInput")
    wvT = nc.dram_tensor("wvT", [D, FPC], MT, kind="ExternalInput")
    bv = nc.dram_tensor("bv", [FPC], MT, kind="ExternalInput")
    cosr = nc.dram_tensor("cosr", [P, S], F16, kind="ExternalInput")
    sinr = nc.dram_tensor("sinr", [P, S], F16, kind="ExternalInput")
    biasT = nc.dram_tensor("biasT", [S, S], F16, kind="ExternalInput")
    owT = nc.dram_tensor("owT", [FPC, D], MT, kind="ExternalInput")
    outT = nc.dram_tensor("outT", [D, S], F32, kind="ExternalOutput")

    hT_r = hT.ap().rearrange("(o p) t -> p o t", p=P)
    w4_r = w4.ap().rearrange("(o p) f -> p o f", p=P)
    wv_r = wvT.ap().rearrange("(o p) f -> p o f", p=P)
    ow_r = owT.ap().rearrange("(o p) f -> p o f", p=P)
    b4_r = b4.ap().rearrange("(o p) -> p o", p=P)
    bias_r = biasT.ap().rearrange("(kc p) q -> p kc q", p=P)

    with tile.TileContext(nc) as tc:
        with (
            tc.tile_pool(name="cst", bufs=1) as cst,
            tc.tile_pool(name="pAB", bufs=1) as pAB,
            tc.tile_pool(name="dram", bufs=4, space="DRAM") as dpool,
        ):
            identF = cst.tile([P, P], MT)
            make_identity(nc, identF)
            ones1 = cst.tile([1, P], MT)
            nc.vector.memset(ones1[:], 1.0)
            b4_sb = cst.tile([P, 2 * FPC // P], F32)
            bv_sb = cst.tile([1, FPC], MT)
            eshift = cst.tile([P, 1], F32)
            nc.vector.memset(eshift[:], -12.0)
            perm_sb = cst.tile([P, P], MT)

            # persistent phase A->B products
            qk_sb = pAB.tile([P, 2 * FT, S], MT)          # slots: Q ft 0..FT-1, K ft FT..2FT-1
            v_sb = pAB.tile([P, KC, HPC, 66], MT)          # col 64 = ones
            bias_sb = pAB.tile([P, 2, KC, NQH], F16)       # both q-halves

            nc.vector.memset(v_sb[:, :, :, 64:65], 1.0)

            # ---------------- Phase A: projections + rope ----------------
            with (
                tc.tile_pool(name="pA", bufs=1) as pA,
                tc.tile_pool(name="pAw", bufs=2) as pAw,
                tc.tile_pool(name="psA", bufs=2, space="PSUM") as ppsA,
            ):
                first_wa = {}
                for half in range(2):
                    tsl = slice(half * NQH, (half + 1) * NQH)
                    if half == 0:
                        # hoist the first weight tile to the very front of
                        # the sync queue so the first matmul can start as
                        # soon as the first hidden-state chunk lands
                        wa0 = pAw.tile([P, DC, P], MT, tag="wA")
                        nc.sync.dma_start(wa0[:], w4_r[:, :, 0:P])
                        first_wa[(0, 0)] = wa0
                    cos_sb = pA.tile([P, NQH], F16, tag="cos", bufs=2)
                    nc.gpsimd.dma_start(cos_sb[:], cosr.ap()[:, tsl])
                    sin_sb = pA.tile([P, NQH], F16, tag="sin", bufs=2)
                    nc.gpsimd.dma_start(sin_sb[:], sinr.ap()[:, tsl])
                    if half == 0:
                        nc.gpsimd.dma_start(b4_sb[:], b4_r)
                        nc.gpsimd.dma_start(bv_sb[:], bv.ap()[None, :])
                        nc.gpsimd.dma_start(perm_sb[:], permM.ap())
                    h_sb = pA.tile([P, DC, NQH], MT, tag="hT", bufs=2)
                    for dc in range(DC):
                        nc.sync.dma_start(h_sb[:, dc], hT_r[:, dc, tsl])

                    # Q/K projections; RoPE rotate_half via a permutation
                    # matmul on the biased projection (rot(q + bq) includes
                    # the rotated bias automatically)
                    for qk in range(2):            # 0 = Q, 1 = K
                        for ft in range(FT):
                            fcol = qk * FPC + ft * P
                            wa = first_wa.pop((qk, ft), None) if half == 0 else None
                            if wa is None:
                                wa = pAw.tile([P, DC, P], MT, tag="wA")
                                nc.sync.dma_start(wa[:], w4_r[:, :, fcol:fcol + P])
                            bcol = qk * FT + ft
                            for tq in range(NQH // NTA):
                                qsl = slice(tq * NTA, (tq + 1) * NTA)
                                pa = ppsA.tile([P, NTA], F32, tag="pa", name="pa")
                                for dc in range(DC):
                                    nc.tensor.matmul(pa[:], wa[:, dc], h_sb[:, dc, qsl],
                                                     start=(dc == 0), stop=(dc == DC - 1))
                                ub = pAw.tile([P, NTA], F16, tag="ub")
                                ubi = nc.scalar.activation(
                                    ub[:], pa[:], AF.Identity,
                                    bias=b4_sb[:, bcol:bcol + 1])
                                if qk == 0 and ft == 2 and tq == 0:
                                    # exp(bias) load for this half rides the
                                    # Activation HWDGE queue, pinned behind a
                                    # mid-phase-A op so its 4MB of HBM traffic
                                    # cannot starve the startup-critical loads
                                    bd = nc.scalar.dma_start(
                                        bias_sb[:, half], bias_r[:, :, tsl])
                                    add_dep_helper(bd.ins, ubi.ins, sync=False,
                                                   reason="delay bias load")
                                pr = ppsA.tile([P, NTA], F32, tag="pr", name="pr")
                                nc.tensor.matmul(pr[:], perm_sb[:], ub[:],
                                                 start=True, stop=True)
                                tca = pAw.tile([P, NTA], F16, tag="tca")
                                nc.vector.tensor_mul(tca[:], ub[:], cos_sb[:, qsl])
                                tcb = pAw.tile([P, NTA], F16, tag="tcb")
                                nc.vector.tensor_mul(tcb[:], pr[:], sin_sb[:, qsl])
                                dst = qk_sb[:, qk * FT + ft, half * NQH + tq * NTA:
                                            half * NQH + (tq + 1) * NTA]
                                nc.vector.tensor_add(dst, tca[:], tcb[:])

                    # big attention-bias load for this half rides the
                    # Activation HWDGE queue, emitted only now so its HBM
                    # traffic does not starve the startup-critical loads
                    nc.scalar.dma_start(bias_sb[:, half], bias_r[:, :, tsl])

                    # V for this half: t-tiles within half
                    wvs = pA.tile([P, DC, FPC], MT, tag="wV")
                    nc.gpsimd.dma_start(wvs[:], wv_r)
                    for tt in range(NQH // P):
                        gt = half * (NQH // P) + tt            # global t-tile = k-chunk
                        pv = ppsA.tile([P, FPC], F32, tag="pv", name="pv")
                        for dc in range(DC):
                            nc.tensor.matmul(pv[:], h_sb[:, dc, tt * P:(tt + 1) * P],
                                             wvs[:, dc], start=(dc == 0), stop=False)
                        nc.tensor.matmul(pv[:], ones1[:], bv_sb[:], start=False, stop=True)
                        nc.vector.tensor_copy(v_sb[:, gt, :, 0:64], pv[:])

            with tc.tile_pool(name="pBC", bufs=1) as pBC:
                ctxT = pBC.tile([P, FT, S], MT)            # normalized ctx, f-major
                ow_sb = pBC.tile([P, FT, D], MT)
                nc.sync.dma_start(ow_sb[:], ow_r)

                # ---------------- Phase B: attention ----------------
                with (
                    tc.tile_pool(name="pB", bufs=2) as pB,
                    tc.tile_pool(name="psB", bufs=1, space="PSUM") as ppsB,
                ):
                    for qh in range(2):
                        qoff = qh * NQH
                        for hp in range(HPC // 2):
                            cps = []
                            for i in range(2):
                                ct = ppsB.tile([P, NQH], F32, tag=f"ctx{i}",
                                               name=f"ctx{i}")
                                cps.append(ct[:65, :])
                            prev = None
                            for kc in range(KC):
                                for qc in range(2):
                                    csl = slice(qc * NQ, (qc + 1) * NQ)
                                    psS = ppsB.tile([P, 2, NQ], F32, tag="ps",
                                                    name="psS", bufs=2)
                                    prev_mm = None
                                    for hi in range(2):
                                        h = 2 * hp + hi
                                        base = 64 * (h % 2)
                                        ft = h // 2
                                        ksl = qk_sb[base:base + 64, FT + ft,
                                                    kc * P:(kc + 1) * P]
                                        qsl = qk_sb[base:base + 64, ft,
                                                    qoff + qc * NQ:
                                                    qoff + (qc + 1) * NQ]
                                        mm = nc.tensor.matmul(psS[:, hi], ksl,
                                                              qsl, start=True,
                                                              stop=True)
                                        if prev_mm is not None:
                                            add_dep_helper(
                                                mm.ins, prev_mm.ins, sync=False,
                                                reason="scores row-group pairing")
                                        prev_mm = mm
                                    u2e = pB.tile([P, 2, NQ], F16, tag="u2e",
                                                  bufs=3)
                                    nc.scalar.activation(u2e[:], psS[:], AF.Exp,
                                                         bias=eshift[:])
                                    # exp(s+b) = exp(s)*exp(b): bias applied as
                                    # a VectorE multiply (bias_sb holds exp(b))
                                    u2 = pB.tile([P, 2, NQ], F16, tag="u2", bufs=3)
                                    for hi in range(2):
                                        nc.vector.tensor_mul(
                                            u2[:, hi], u2e[:, hi],
                                            bias_sb[:, qh, kc, csl])
                                    # software-pipeline: PV lags one step so the
                                    # PE never waits on this step's exp
                                    if prev is not None:
                                        pkc, pqc, pu = prev
                                        psl = slice(pqc * NQ, (pqc + 1) * NQ)
                                        for hi in range(2):
                                            h = 2 * hp + hi
                                            nc.tensor.matmul(
                                                cps[hi][:, psl],
                                                v_sb[:, pkc, h, 0:65],
                                                pu[:, hi],
                                                start=(pkc == 0),
                                                stop=(pkc == KC - 1))
                                    prev = (kc, qc, u2)
                            # finalize: evacuate ctx PSUM -> SBUF (fp16), then
                            # reciprocal of the denominator row via a [32,32]
                            # DRAM-roundtrip reshape, normalize on VectorE.
                            # qc0's ctx columns are already complete here, so
                            # their evacuation overlaps the trailing PV.
                            cus = []
                            for hi in range(2):
                                cu = pB.tile([65, NQH], F16, tag=f"cu{hi}")
                                nc.vector.tensor_copy(cu[:, 0:NQ], cps[hi][:, 0:NQ])
                                cus.append(cu)
                            pkc, pqc, pu = prev
                            psl = slice(pqc * NQ, (pqc + 1) * NQ)
                            for hi in range(2):
                                h = 2 * hp + hi
                                nc.tensor.matmul(cps[hi][:, psl],
                                                 v_sb[:, pkc, h, 0:65],
                                                 pu[:, hi],
                                                 start=False, stop=True)
                            for hi in range(2):
                                nc.vector.tensor_copy(cus[hi][:, NQ:NQH],
                                                      cps[hi][:, NQ:NQH])
                            rscrs, rsqs, rrecs, rscr2s, rbs = [], [], [], [], []
                            for hi in range(2):
                                rscr = dpool.tile([NQH], F16)
                                nc.gpsimd.dma_start(rscr[None, :],
                                                    cus[hi][64:65, :])
                                rscrs.append(rscr)
                            for hi in range(2):
                                rsq = pB.tile([32, NQH // 32], F16, tag=f"rsq{hi}")
                                nc.gpsimd.dma_start(
                                    rsq[:], rscrs[hi].rearrange("(a b) -> a b", a=32))
                                rsqs.append(rsq)
                            for hi in range(2):
                                rrec = pB.tile([32, NQH // 32], F16, tag=f"rrec{hi}")
                                with nc.allow_low_precision(
                                        reason="softmax denom fits fp16"):
                                    nc.vector.reciprocal(rrec[:], rsqs[hi][:])
                                rrecs.append(rrec)
                            for hi in range(2):
                                rscr2 = dpool.tile([NQH], F16)
                                nc.gpsimd.dma_start(
                                    rscr2.rearrange("(a b) -> a b", a=32), rrecs[hi][:])
                                rscr2s.append(rscr2)
                            for hi in range(2):
                                rb = pB.tile([64, NQH], F16, tag=f"rb{hi}")
                                nc.gpsimd.dma_start(rb[:],
                                                    rscr2s[hi].partition_broadcast(64))
                                rbs.append(rb)
                            for hi in range(2):
                                h = 2 * hp + hi
                                base = 64 * (h % 2)
                                ft = h // 2
                                nc.vector.tensor_mul(
                                    ctxT[base:base + 64, ft, qoff:qoff + NQH],
                                    cus[hi][0:64, :], rbs[hi][:])

                # ---------------- Phase C: output projection ----------------
                with (
                    tc.tile_pool(name="pC", bufs=4) as pC,
                    tc.tile_pool(name="psC", bufs=4, space="PSUM") as ppsC,
                ):
                    ci = 0
                    for tq in range(S // NT):
                        for ot in range(D // P):
                            tsl = slice(tq * NT, (tq + 1) * NT)
                            po = ppsC.tile([P, NT], F32, tag="po", name="po")
                            for fc in range(FT):
                                nc.tensor.matmul(po[:],
                                                 ow_sb[:, fc, ot * P:(ot + 1) * P],
                                                 ctxT[:, fc, tsl],
                                                 start=(fc == 0), stop=(fc == FT - 1))
                            o_sb = pC.tile([P, NT], F32, tag="oT")
                            if ci % 2 == 0:
                                nc.scalar.copy(o_sb[:], po[:])
                                nc.sync.dma_start(
                                    outT.ap()[ot * P:(ot + 1) * P, tsl], o_sb[:])
                            else:
                                nc.vector.tensor_copy(o_sb[:], po[:])
                                nc.scalar.dma_start(
                                    outT.ap()[ot * P:(ot + 1) * P, tsl], o_sb[:])
                            ci += 1

    nc.compile()
    return nc


def make_core_inputs(hidden_states, attention_bias, rope_cos, rope_sin, head_mask,
                     qkv_w, qkv_b, o_w, S=2048, D=1024):
    """Host-side sharding + layout preparation. Returns list of 8 input dicts."""
    f32 = np.float32
    f16 = np.float16
    mt = f16
    hidden_states = np.asarray(hidden_states, f32)
    attention_bias = np.asarray(attention_bias, f32)
    rope_cos = np.asarray(rope_cos, f32)
    rope_sin = np.asarray(rope_sin, f32)
    head_mask = np.asarray(head_mask, f32).reshape(-1)
    qkv_w = np.asarray(qkv_w, f32)
    qkv_b = np.asarray(qkv_b, f32)
    o_w = np.asarray(o_w, f32)

    B = hidden_states.shape[0]
    FPC = HPC * 64
    F = H * 64  # qkv feature dim (row-section size of qkv_w)

    def rot_rows(w):
        # rows indexed by f = hl*64 + d; rot(q)[d] = -q[d+32] (d<32) else q[d-32]
        w = w.reshape(HPC, 64, -1) if w.ndim == 2 else w.reshape(HPC, 64)
        lo, hi = w[:, 0:32], w[:, 32:64]
        out = np.concatenate([-hi, lo], axis=1)
        return out.reshape(HPC * 64, -1) if out.ndim == 3 else out.reshape(HPC * 64)

    cos_t = rope_cos[0, :, 0, :].T.astype(f32)     # [64, S]
    sin_t = rope_sin[0, :, 0, :].T.astype(f32)
    cosr = np.concatenate([cos_t, cos_t], axis=0)  # [128, S]
    sinr = np.concatenate([sin_t, sin_t], axis=0)

    in_maps = []
    for c in range(8):
        b, g = divmod(c, G)
        fs = slice(g * FPC, (g + 1) * FPC)
        wq = qkv_w[F * 0:F * 1][fs]
        wk = qkv_w[F * 1:F * 2][fs]
        wv = qkv_w[F * 2:F * 3][fs].copy()
        bq = qkv_b[F * 0:F * 1][fs]
        bk = qkv_b[F * 1:F * 2][fs]
        bvv = qkv_b[F * 2:F * 3][fs].copy()
        mask = head_mask[g * HPC:(g + 1) * HPC]
        wv *= np.repeat(mask, 64)[:, None]
        bvv *= np.repeat(mask, 64)
        w4 = np.concatenate([wq.T, wk.T], axis=1)  # [D, 2*FPC]
        b4 = np.concatenate([bq, bk])
        # lhsT for r = rot(u): out[p] = -u[p+32] (p%64<32) else u[p-32],
        # block-diagonal per 64-row head
        permM = np.zeros((128, 128), f32)
        for hb in (0, 64):
            for dd in range(32):
                permM[hb + dd + 32, hb + dd] = -1.0
                permM[hb + dd, hb + dd + 32] = 1.0
        bT = np.ascontiguousarray(attention_bias[b, 0].T)
        m = {
            "hT": np.ascontiguousarray(hidden_states[b].T).astype(mt),
            "w4": np.ascontiguousarray(w4).astype(mt),
            "b4": np.ascontiguousarray(b4),
            "permM": permM.astype(mt),
            "wvT": np.ascontiguousarray(wv.T).astype(mt),
            "bv": np.ascontiguousarray(bvv).astype(mt),
            "cosr": np.ascontiguousarray(cosr).astype(f16),
            "sinr": np.ascontiguousarray(sinr).astype(f16),
            "biasT": np.exp(bT).astype(f16),
            "owT": np.ascontiguousarray(o_w[:, g * FPC:(g + 1) * FPC].T).astype(mt),
        }
        in_maps.append(m)
    return in_maps


def kernel(hidden_states, attention_bias, rope_cos, rope_sin, head_mask,
           qkv_w, qkv_b, o_w, o_b, **_unused):
    from concourse.bass_utils import run_bass_kernel_spmd

    B, S, D = hidden_states.shape
    if "nc" not in _CACHE:
        _CACHE["nc"] = build_nc(S=S, D=D)
    nc = _CACHE["nc"]

    in_maps = make_core_inputs(hidden_states, attention_bias, rope_cos, rope_sin,
                               head_mask, qkv_w, qkv_b, o_w, S=S, D=D)
    res = run_bass_kernel_spmd(nc, in_maps, list(range(8)))
    _CACHE["last_results"] = res

    o_b = np.asarray(o_b, np.float32)
    out = np.empty((B, S, D), np.float32)
    for b in range(B):
        acc = res.results[2 * b]["outT"].T + res.results[2 * b + 1]["outT"].T
        out[b] = acc + o_b[None, :]
    return out
